# revision 7
# baseline (speedup 1.0000x reference)
"""Multi-head GAT layer (2 heads, sum-merged) on 8 TRN2 NeuronCores.

Strategy: edges are sharded by destination node (12500 dsts per core), so
the segment softmax and scatter-sum are entirely core-local (no
collectives). Node features and weights are replicated; every core
computes the full projected-source table Z = [z | s_src] once, then
processes only its own edges via indirect-DMA gathers. All data-dependent
structure (edge->slot assignment, output rows) is carried in index
tensors, so the compiled program is identical across cores (SPMD).
"""

import numpy as np
import ml_dtypes

import concourse.bass as bass
import concourse.bacc as bacc
import concourse.mybir as mybir
import concourse.tile as tile
from concourse.bass_utils import run_bass_kernel_spmd

F32 = mybir.dt.float32
BF16 = mybir.dt.bfloat16
I32 = mybir.dt.int32

IN = 128          # input feature dim
OUT = 64          # output feature dim per head
H = 2             # heads
ZC = IN + 4       # z-row: 128 z cols + 2 s_src cols + 2 pad = 132
NCORES = 8
K = 8             # edge chunks (of 128) per supertile
CAP = 128 * K     # edge capacity per supertile

N_SRC = 100000
N_DST = 100000
NDST_C = N_DST // NCORES            # 12500 dsts per core
SRC_TILES = 784                     # 784*128 = 100352 >= N_SRC
SRC_PAD = SRC_TILES * 128
SRC_GROUP = 8                       # src tiles per load group (98 groups)
DST_TILES = 98                      # 98*128 = 12544 >= NDST_C
DST_PAD = DST_TILES * 128
DST_GROUP = 7                       # dst tiles per load group (14 groups)
OUT_ROWS = DST_PAD + 128            # trailing 128 rows catch garbage


def _pack_core(src_c, dst_local, dst_pad, cap=None, k=None):
    cap = CAP if cap is None else cap
    k = K if k is None else k
    """Pack one core's edges (dst-sorted) into supertiles.

    Returns list of per-tile dicts with slot-index arrays. Each supertile
    holds whole dst segments only, <= cap edges, dst span <= 128.
    """
    order = np.argsort(dst_local, kind="stable")
    s = np.ascontiguousarray(src_c[order])
    d = np.ascontiguousarray(dst_local[order])
    n = len(d)
    tiles = []
    if n:
        starts = np.flatnonzero(np.r_[True, np.diff(d) != 0])
        ends = np.r_[starts[1:], n]
        segd = d[starts]
        nseg = len(starts)
        cur = 0
        while cur < nseg:
            d0 = int(segd[cur])
            elo = int(starts[cur])
            assert int(ends[cur]) - elo <= cap, "segment larger than supertile"
            hi = cur
            while (
                hi + 1 < nseg
                and int(ends[hi + 1]) - elo <= cap
                and int(segd[hi + 1]) - d0 < 128
            ):
                hi += 1
            tiles.append((d0, elo, int(ends[hi])))
            cur = hi + 1

    out = []
    for d0, elo, ehi in tiles:
        cnt = ehi - elo
        e = np.arange(cnt)
        p, j = e // k, e % k
        gidx = np.zeros((128, k), np.int32)
        dstrel = np.full((128, k), -1, np.int32)
        sdidx = np.zeros((128, k), np.int32)
        gidx[p, j] = s[elo:ehi]
        dstrel[p, j] = d[elo:ehi] - d0
        sdidx[p, j] = d[elo:ehi]
        span = int(d[ehi - 1]) - d0 + 1
        rows = d0 + np.arange(128, dtype=np.int32)
        rows[span:] = dst_pad + np.arange(span, 128, dtype=np.int32)
        out.append((gidx, dstrel, sdidx, rows))
    return out


def _pack_all(src_idx, dst_idx):
    """Pack every core's edges; pad to a common supertile count T."""
    ncores, ndst_c, dst_pad = NCORES, NDST_C, DST_PAD
    per_core = []
    core_of = dst_idx // ndst_c
    for c in range(ncores):
        m = core_of == c
        per_core.append(_pack_core(src_idx[m], dst_idx[m] - c * ndst_c, dst_pad))
    T = max(len(t) for t in per_core)
    eidx = np.zeros((ncores, T, 128, 3 * K + 1), np.int32)
    dummy_rows = dst_pad + np.arange(128, dtype=np.int32)
    for c in range(ncores):
        for ti in range(T):
            if ti < len(per_core[c]):
                gidx, dstrel, sdidx, rows = per_core[c][ti]
            else:
                gidx = np.zeros((128, K), np.int32)
                dstrel = np.full((128, K), -1, np.int32)
                sdidx = np.zeros((128, K), np.int32)
                rows = dummy_rows
            eidx[c, ti, :, 0:K] = gidx
            eidx[c, ti, :, K:2 * K] = dstrel
            eidx[c, ti, :, 2 * K:3 * K] = sdidx
            eidx[c, ti, :, 3 * K] = rows
    return eidx, T


def _build_program(T):
    nc = bacc.Bacc("TRN2", target_bir_lowering=False, debug=False,
                   num_devices=NCORES)
    hsT = nc.dram_tensor("hsrcT", [128, SRC_PAD], BF16, kind="ExternalInput").ap()
    hdT = nc.dram_tensor("hdstT", [128, DST_PAD], BF16, kind="ExternalInput").ap()
    wsr = nc.dram_tensor("wsrc", [128, ZC], BF16, kind="ExternalInput").ap()
    wds = nc.dram_tensor("wdst", [128, 2], BF16, kind="ExternalInput").ap()
    eix = nc.dram_tensor("eidx", [T, 128, 3 * K + 1], I32, kind="ExternalInput").ap()
    Z = nc.dram_tensor("Z", [SRC_PAD, ZC], F32, kind="Internal").ap()
    SD = nc.dram_tensor("SD", [DST_PAD, 2], F32, kind="Internal").ap()
    out = nc.dram_tensor("out", [OUT_ROWS, OUT], F32, kind="ExternalOutput").ap()

    with tile.TileContext(nc) as tc:
        with (
            tc.tile_pool(name="const", bufs=1) as cpool,
            tc.tile_pool(name="pa", bufs=3) as pa_pool,
            tc.tile_pool(name="pz", bufs=3) as pz_pool,
            tc.tile_pool(name="sda", bufs=1) as sd_pool,
            tc.tile_pool(name="psA", bufs=3, space="PSUM") as psA_pool,
            tc.tile_pool(name="psD", bufs=2, space="PSUM") as psD_pool,
            tc.tile_pool(name="psB", bufs=2, space="PSUM") as psB_pool,
            tc.tile_pool(name="ei", bufs=3) as ei_pool,
            tc.tile_pool(name="zg", bufs=3) as zg_pool,
            tc.tile_pool(name="oht", bufs=3) as oht_pool,
            tc.tile_pool(name="wt", bufs=3) as w_pool,
            tc.tile_pool(name="fl", bufs=3) as f_pool,
        ):
            wsrc_t = cpool.tile([128, ZC], BF16)
            nc.sync.dma_start(out=wsrc_t[:], in_=wsr[:, :])
            wdst_t = cpool.tile([128, 2], BF16)
            nc.sync.dma_start(out=wdst_t[:], in_=wds[:, :])
            iota_t = cpool.tile([128, K * 128], I32)
            nc.gpsimd.iota(iota_t[:], [[0, K], [1, 128]], channel_multiplier=0)

            # ---- Phase A: Z = [z | s_src] for all src nodes ----
            for g in range(SRC_TILES // SRC_GROUP):
                hT = pa_pool.tile([128, SRC_GROUP * 128], BF16)
                nc.sync.dma_start(
                    out=hT[:],
                    in_=hsT[:, g * SRC_GROUP * 128:(g + 1) * SRC_GROUP * 128])
                zbig = pz_pool.tile([128, SRC_GROUP * ZC], F32)
                for j in range(SRC_GROUP):
                    ps = psA_pool.tile([128, ZC], F32)
                    nc.tensor.matmul(
                        out=ps[:], lhsT=hT[:, j * 128:(j + 1) * 128],
                        rhs=wsrc_t[:], start=True, stop=True)
                    nc.vector.tensor_copy(
                        out=zbig[:, j * ZC:(j + 1) * ZC], in_=ps[:])
                rows = slice(g * SRC_GROUP * 128, (g + 1) * SRC_GROUP * 128)
                nc.sync.dma_start(
                    out=Z[rows, :].rearrange("(j p) c -> p j c", p=128),
                    in_=zbig[:].rearrange("p (j c) -> p j c", c=ZC))

            # ---- Phase A: s_dst for this core's dst shard ----
            sdall = sd_pool.tile([128, 2 * DST_TILES], F32)
            for g in range(DST_TILES // DST_GROUP):
                hTd = pa_pool.tile([128, DST_GROUP * 128], BF16, tag="hTd")
                nc.sync.dma_start(
                    out=hTd[:],
                    in_=hdT[:, g * DST_GROUP * 128:(g + 1) * DST_GROUP * 128])
                for j in range(DST_GROUP):
                    t = g * DST_GROUP + j
                    psd = psD_pool.tile([128, 2], F32, tag="psd")
                    nc.tensor.matmul(
                        out=psd[:], lhsT=hTd[:, j * 128:(j + 1) * 128],
                        rhs=wdst_t[:], start=True, stop=True)
                    nc.vector.tensor_copy(
                        out=sdall[:, t * 2:(t + 1) * 2], in_=psd[:])
            nc.sync.dma_start(
                out=SD[:, :].rearrange("(t p) c -> p t c", p=128),
                in_=sdall[:].rearrange("p (t c) -> p t c", c=2))

            # ---- Phase B: edge supertiles ----
            for t in range(T):
                ei = ei_pool.tile([128, 3 * K + 1], I32)
                nc.sync.dma_start(out=ei[:], in_=eix[t, :, :])
                zg = zg_pool.tile([128, K * ZC], F32)
                for j in range(K):
                    nc.gpsimd.indirect_dma_start(
                        out=zg[:, j * ZC:(j + 1) * ZC], out_offset=None,
                        in_=Z[:, :],
                        in_offset=bass.IndirectOffsetOnAxis(
                            ap=ei[:, j:j + 1], axis=0))
                sdg = w_pool.tile([128, K * 2], F32, tag="sdg")
                for j in range(K):
                    nc.gpsimd.indirect_dma_start(
                        out=sdg[:, j * 2:(j + 1) * 2], out_offset=None,
                        in_=SD[:, :],
                        in_offset=bass.IndirectOffsetOnAxis(
                            ap=ei[:, 2 * K + j:2 * K + j + 1], axis=0))
                oht = oht_pool.tile([128, K * 128], F32)
                nc.vector.tensor_tensor(
                    out=oht[:],
                    in0=ei[:, K:2 * K].to_broadcast([128, K, 128]),
                    in1=iota_t[:].rearrange("p (k q) -> p k q", q=128),
                    op=mybir.AluOpType.is_equal)
                zg3 = zg[:].rearrange("p (j c) -> p j c", c=ZC)
                st = w_pool.tile([128, K * 2], F32, tag="st")
                nc.vector.tensor_tensor(
                    out=st[:].rearrange("p (j c) -> p j c", c=2),
                    in0=zg3[:, :, IN:IN + 2],
                    in1=sdg[:].rearrange("p (j c) -> p j c", c=2),
                    op=mybir.AluOpType.add)
                st2 = w_pool.tile([128, K * 2], F32, tag="st2")
                nc.vector.tensor_scalar_mul(out=st2[:], in0=st[:], scalar1=0.01)
                nc.vector.tensor_tensor(
                    out=st[:], in0=st[:], in1=st2[:], op=mybir.AluOpType.max)
                wt = w_pool.tile([128, K * 2], F32, tag="wt")
                nc.scalar.activation(
                    out=wt[:], in_=st[:], func=mybir.ActivationFunctionType.Exp)
                wt3 = wt[:].rearrange("p (j c) -> p j c", c=2)
                for h in range(H):
                    nc.vector.tensor_tensor(
                        out=zg3[:, :, h * OUT:(h + 1) * OUT],
                        in0=zg3[:, :, h * OUT:(h + 1) * OUT],
                        in1=wt3[:, :, h:h + 1].to_broadcast([128, K, OUT]),
                        op=mybir.AluOpType.mult)
                nc.vector.tensor_copy(out=zg3[:, :, IN:IN + 2], in_=wt3[:, :, :])
                ps = psB_pool.tile([128, ZC], F32)
                for j in range(K):
                    nc.tensor.matmul(
                        out=ps[:], lhsT=oht[:, j * 128:(j + 1) * 128],
                        rhs=zg[:, j * ZC:(j + 1) * ZC],
                        start=(j == 0), stop=(j == K - 1))
                den = f_pool.tile([128, 2], F32, tag="den")
                nc.vector.tensor_scalar_max(
                    out=den[:], in0=ps[:, IN:IN + 2], scalar1=1e-30)
                rec = f_pool.tile([128, 2], F32, tag="rec")
                nc.vector.reciprocal(out=rec[:], in_=den[:])
                o0 = f_pool.tile([128, OUT], F32, tag="o0")
                nc.vector.tensor_scalar_mul(
                    out=o0[:], in0=ps[:, 0:OUT], scalar1=rec[:, 0:1])
                ot = f_pool.tile([128, OUT], F32, tag="ot")
                nc.vector.tensor_scalar_mul(
                    out=ot[:], in0=ps[:, OUT:2 * OUT], scalar1=rec[:, 1:2])
                nc.vector.tensor_add(out=ot[:], in0=ot[:], in1=o0[:])
                nc.gpsimd.indirect_dma_start(
                    out=out[:, :],
                    out_offset=bass.IndirectOffsetOnAxis(
                        ap=ei[:, 3 * K:3 * K + 1], axis=0),
                    in_=ot[:], in_offset=None)

    nc.compile()
    return nc


def _prep_inputs(h_src, h_dst, W_src, W_dst, a_w, src_idx, dst_idx):
    """Host-side sharding/layout prep. Returns in_maps for the 8 cores."""
    hs = np.zeros((SRC_PAD, IN), np.float32)
    hs[:N_SRC] = h_src
    hsrcT = np.ascontiguousarray(hs.T.astype(ml_dtypes.bfloat16))

    # wsrc: [IN, ZC] = [ W[h,o,d] at col h*OUT+o | w~_s | pad ]
    wsr = np.zeros((IN, ZC), np.float32)
    wsr[:, :H * OUT] = W_src.reshape(H * OUT, IN).T
    a_s, a_d = a_w[:, :OUT], a_w[:, OUT:]
    wsr[:, H * OUT:H * OUT + H] = np.einsum("hod,ho->dh", W_src, a_s)
    wsr = wsr.astype(ml_dtypes.bfloat16)
    wds = np.einsum("hod,ho->dh", W_dst, a_d).astype(ml_dtypes.bfloat16)

    eidx, T = _pack_all(src_idx, dst_idx)

    in_maps = []
    for c in range(NCORES):
        hd = np.zeros((DST_PAD, IN), np.float32)
        hd[:NDST_C] = h_dst[c * NDST_C:(c + 1) * NDST_C]
        hdstT = np.ascontiguousarray(hd.T.astype(ml_dtypes.bfloat16))
        in_maps.append({
            "hsrcT": hsrcT,
            "hdstT": hdstT,
            "wsrc": wsr,
            "wdst": wds,
            "eidx": eidx[c],
        })
    return in_maps, T


def _run(inputs, trace=False):
    in_maps, T = _prep_inputs(**inputs)
    nc = _build_program(T)
    res = run_bass_kernel_spmd(
        nc, in_maps, core_ids=list(range(NCORES)), trace=trace)
    parts = [res.results[c]["out"][:NDST_C] for c in range(NCORES)]
    return np.concatenate(parts, axis=0), res


def kernel(**inputs):
    out, _ = _run(inputs, trace=False)
    return out


# revision 13
# speedup vs baseline: 1.3078x; 1.3078x over previous
"""Multi-head GAT layer (2 heads, sum-merged) on 8 TRN2 NeuronCores.

Strategy: edges are sharded by destination node (12500 dsts per core), so
the segment softmax and scatter-sum are entirely core-local (no
collectives). Node features and weights are replicated; every core
computes the full projected-source table Z = [z | s_src] once, then
processes only its own edges via indirect-DMA gathers. All data-dependent
structure (edge->slot assignment, output rows) is carried in index
tensors, so the compiled program is identical across cores (SPMD).
"""

import numpy as np
import ml_dtypes

import concourse.bass as bass
import concourse.bacc as bacc
import concourse.mybir as mybir
import concourse.tile as tile
from concourse.bass_utils import run_bass_kernel_spmd

F32 = mybir.dt.float32
BF16 = mybir.dt.bfloat16
I32 = mybir.dt.int32

IN = 128          # input feature dim
OUT = 64          # output feature dim per head
H = 2             # heads
ZC = IN + 4       # z-row: 128 z cols + 2 s_src cols + 2 pad = 132
NCORES = 8
K = 8             # edge chunks (of 128) per supertile
CAP = 128 * K     # edge capacity per supertile

N_SRC = 100000
N_DST = 100000
NDST_C = N_DST // NCORES            # 12500 dsts per core
SRC_TILES = 784                     # 784*128 = 100352 >= N_SRC
SRC_PAD = SRC_TILES * 128
SRC_GROUP = 8                       # src tiles per load group (98 groups)
DST_TILES = 98                      # 98*128 = 12544 >= NDST_C
DST_PAD = DST_TILES * 128
DST_GROUP = 7                       # dst tiles per load group (14 groups)
OUT_ROWS = DST_PAD + 128            # trailing 128 rows catch garbage


def _pack_core(src_c, dst_local, dst_pad, cap=None, k=None):
    cap = CAP if cap is None else cap
    k = K if k is None else k
    """Pack one core's edges (dst-sorted) into supertiles.

    Returns list of per-tile dicts with slot-index arrays. Each supertile
    holds whole dst segments only, <= cap edges, dst span <= 128.
    """
    order = np.argsort(dst_local, kind="stable")
    s = np.ascontiguousarray(src_c[order])
    d = np.ascontiguousarray(dst_local[order])
    n = len(d)
    tiles = []
    if n:
        starts = np.flatnonzero(np.r_[True, np.diff(d) != 0])
        ends = np.r_[starts[1:], n]
        segd = d[starts]
        nseg = len(starts)
        cur = 0
        while cur < nseg:
            d0 = int(segd[cur])
            elo = int(starts[cur])
            assert int(ends[cur]) - elo <= cap, "segment larger than supertile"
            hi = cur
            while (
                hi + 1 < nseg
                and int(ends[hi + 1]) - elo <= cap
                and int(segd[hi + 1]) - d0 < 128
            ):
                hi += 1
            tiles.append((d0, elo, int(ends[hi])))
            cur = hi + 1

    out = []
    for d0, elo, ehi in tiles:
        cnt = ehi - elo
        e = np.arange(cnt)
        p, j = e // k, e % k
        gidx = np.zeros((128, k), np.int32)
        dstrel = np.full((128, k), -1, np.int32)
        sdidx = np.zeros((128, k), np.int32)
        gidx[p, j] = s[elo:ehi]
        dstrel[p, j] = d[elo:ehi] - d0
        sdidx[p, j] = d[elo:ehi]
        span = int(d[ehi - 1]) - d0 + 1
        rows = d0 + np.arange(128, dtype=np.int32)
        rows[span:] = dst_pad + np.arange(span, 128, dtype=np.int32)
        out.append((gidx, dstrel, sdidx, rows))
    return out


def _pack_all(src_idx, dst_idx):
    """Pack every core's edges; pad to a common supertile count T."""
    ncores, ndst_c, dst_pad = NCORES, NDST_C, DST_PAD
    per_core = []
    core_of = dst_idx // ndst_c
    for c in range(ncores):
        m = core_of == c
        per_core.append(_pack_core(src_idx[m], dst_idx[m] - c * ndst_c, dst_pad))
    T = max(len(t) for t in per_core)
    eidx = np.zeros((ncores, T, 128, 3 * K + 1), np.int32)
    dummy_rows = dst_pad + np.arange(128, dtype=np.int32)
    for c in range(ncores):
        for ti in range(T):
            if ti < len(per_core[c]):
                gidx, dstrel, sdidx, rows = per_core[c][ti]
            else:
                gidx = np.zeros((128, K), np.int32)
                dstrel = np.full((128, K), -1, np.int32)
                sdidx = np.zeros((128, K), np.int32)
                rows = dummy_rows
            eidx[c, ti, :, 0:K] = gidx
            eidx[c, ti, :, K:2 * K] = dstrel
            eidx[c, ti, :, 2 * K:3 * K] = sdidx
            eidx[c, ti, :, 3 * K] = rows
    return eidx, T


def _build_program(T):
    nc = bacc.Bacc("TRN2", target_bir_lowering=False, debug=False,
                   num_devices=NCORES)
    hsT = nc.dram_tensor("hsrcT", [128, SRC_PAD], BF16, kind="ExternalInput").ap()
    hdT = nc.dram_tensor("hdstT", [128, DST_PAD], BF16, kind="ExternalInput").ap()
    wsr = nc.dram_tensor("wsrc", [128, ZC], BF16, kind="ExternalInput").ap()
    wds = nc.dram_tensor("wdst", [128, 2], BF16, kind="ExternalInput").ap()
    eix = nc.dram_tensor("eidx", [T, 128, 3 * K + 1], I32, kind="ExternalInput").ap()
    Z = nc.dram_tensor("Z", [SRC_PAD, ZC], F32, kind="Internal").ap()
    SD = nc.dram_tensor("SD", [OUT_ROWS, 2], F32, kind="Internal").ap()
    out = nc.dram_tensor("out", [OUT_ROWS, OUT], F32, kind="ExternalOutput").ap()

    from concourse.masks import make_identity

    with tile.TileContext(nc) as tc:
        with (
            tc.tile_pool(name="const", bufs=1) as cpool,
            tc.tile_pool(name="pa", bufs=3) as pa_pool,
            tc.tile_pool(name="pz", bufs=3) as pz_pool,
            tc.tile_pool(name="sda", bufs=1) as sd_pool,
            tc.tile_pool(name="psA", bufs=3, space="PSUM") as psA_pool,
            tc.tile_pool(name="psD", bufs=1, space="PSUM") as psD_pool,
            tc.tile_pool(name="psB", bufs=2, space="PSUM") as psB_pool,
            tc.tile_pool(name="psOH", bufs=1, space="PSUM") as psOH_pool,
            tc.tile_pool(name="psSD", bufs=1, space="PSUM") as psSD_pool,
            tc.tile_pool(name="ei", bufs=3) as ei_pool,
            tc.tile_pool(name="zg", bufs=3) as zg_pool,
            tc.tile_pool(name="oht", bufs=3) as oht_pool,
            tc.tile_pool(name="wt", bufs=3) as w_pool,
            tc.tile_pool(name="fl", bufs=3) as f_pool,
        ):
            wsrc_t = cpool.tile([128, ZC], BF16)
            nc.sync.dma_start(out=wsrc_t[:], in_=wsr[:, :])
            wdst_t = cpool.tile([128, 2], BF16)
            nc.sync.dma_start(out=wdst_t[:], in_=wds[:, :])
            iota_t = cpool.tile([128, K * 128], I32)
            nc.gpsimd.iota(iota_t[:], [[0, K], [1, 128]], channel_multiplier=0)
            iop_t = cpool.tile([128, 1], F32)
            nc.gpsimd.iota(iop_t[:], [[0, 1]], channel_multiplier=1,
                           allow_small_or_imprecise_dtypes=True)
            ident_t = cpool.tile([128, 128], F32)
            make_identity(nc, ident_t[:])
            zpad_t = cpool.tile([128, 2], F32)
            nc.gpsimd.memset(zpad_t[:], 0.0)
            nc.sync.dma_start(out=SD[DST_PAD:OUT_ROWS, :], in_=zpad_t[:])

            # ---- Phase A: Z = [z | s_src] for all src nodes ----
            for g in range(SRC_TILES // SRC_GROUP):
                hT = pa_pool.tile([128, SRC_GROUP * 128], BF16)
                nc.sync.dma_start(
                    out=hT[:],
                    in_=hsT[:, g * SRC_GROUP * 128:(g + 1) * SRC_GROUP * 128])
                zbig = pz_pool.tile([128, SRC_GROUP * ZC], F32)
                for j in range(SRC_GROUP):
                    ps = psA_pool.tile([128, ZC], F32)
                    nc.tensor.matmul(
                        out=ps[:], lhsT=hT[:, j * 128:(j + 1) * 128],
                        rhs=wsrc_t[:], start=True, stop=True)
                    nc.vector.tensor_copy(
                        out=zbig[:, j * ZC:(j + 1) * ZC], in_=ps[:])
                rows = slice(g * SRC_GROUP * 128, (g + 1) * SRC_GROUP * 128)
                nc.sync.dma_start(
                    out=Z[rows, :].rearrange("(j p) c -> p j c", p=128),
                    in_=zbig[:].rearrange("p (j c) -> p j c", c=ZC))

            # ---- Phase A: s_dst for this core's dst shard ----
            sdall = sd_pool.tile([128, 2 * DST_TILES], F32)
            for g in range(DST_TILES // DST_GROUP):
                hTd = pa_pool.tile([128, DST_GROUP * 128], BF16, tag="hTd")
                nc.sync.dma_start(
                    out=hTd[:],
                    in_=hdT[:, g * DST_GROUP * 128:(g + 1) * DST_GROUP * 128])
                for j in range(DST_GROUP):
                    t = g * DST_GROUP + j
                    psd = psD_pool.tile([128, 2], F32, tag="psd")
                    nc.tensor.matmul(
                        out=psd[:], lhsT=hTd[:, j * 128:(j + 1) * 128],
                        rhs=wdst_t[:], start=True, stop=True)
                    nc.vector.tensor_copy(
                        out=sdall[:, t * 2:(t + 1) * 2], in_=psd[:])
            nc.sync.dma_start(
                out=SD[0:DST_PAD, :].rearrange("(t p) c -> p t c", p=128),
                in_=sdall[:].rearrange("p (t c) -> p t c", c=2))

            # ---- Phase B: edge supertiles ----
            for t in range(T):
                ei = ei_pool.tile([128, 3 * K + 1], I32)
                nc.sync.dma_start(out=ei[:], in_=eix[t, :, :])
                zg = zg_pool.tile([128, K * ZC], F32)
                for j in range(K):
                    nc.gpsimd.indirect_dma_start(
                        out=zg[:, j * ZC:(j + 1) * ZC], out_offset=None,
                        in_=Z[:, :],
                        in_offset=bass.IndirectOffsetOnAxis(
                            ap=ei[:, j:j + 1], axis=0))
                # s_dst for the tile's 128 dst slots (one gather), then
                # per-edge expansion via one-hot matmuls on the PE.
                sdslot = w_pool.tile([128, 2], F32, tag="sdslot")
                nc.gpsimd.indirect_dma_start(
                    out=sdslot[:], out_offset=None, in_=SD[:, :],
                    in_offset=bass.IndirectOffsetOnAxis(
                        ap=ei[:, 3 * K:3 * K + 1], axis=0))
                drelF = w_pool.tile([128, K], F32, tag="drelF")
                nc.vector.tensor_copy(out=drelF[:], in_=ei[:, K:2 * K])
                sdg = w_pool.tile([128, K * 2], F32, tag="sdg")
                for j in range(K):
                    ps_oh = psOH_pool.tile([128, 128], F32, tag="psoh")
                    nc.tensor.transpose(
                        out=ps_oh[:],
                        in_=drelF[:, j:j + 1].to_broadcast([128, 128]),
                        identity=ident_t[:])
                    oh_s = oht_pool.tile([128, 128], F32, tag="ohs")
                    nc.vector.tensor_tensor(
                        out=oh_s[:], in0=iop_t[:, 0:1].to_broadcast([128, 128]),
                        in1=ps_oh[:], op=mybir.AluOpType.is_equal)
                    ps_sd = psSD_pool.tile([128, 2], F32, tag="pssd")
                    nc.tensor.matmul(out=ps_sd[:], lhsT=oh_s[:],
                                     rhs=sdslot[:], start=True, stop=True)
                    nc.vector.tensor_copy(
                        out=sdg[:, j * 2:(j + 1) * 2], in_=ps_sd[:])
                oht = oht_pool.tile([128, K * 128], F32)
                nc.vector.tensor_tensor(
                    out=oht[:],
                    in0=ei[:, K:2 * K].to_broadcast([128, K, 128]),
                    in1=iota_t[:].rearrange("p (k q) -> p k q", q=128),
                    op=mybir.AluOpType.is_equal)
                zg3 = zg[:].rearrange("p (j c) -> p j c", c=ZC)
                st = w_pool.tile([128, K * 2], F32, tag="st")
                nc.vector.tensor_tensor(
                    out=st[:].rearrange("p (j c) -> p j c", c=2),
                    in0=zg3[:, :, IN:IN + 2],
                    in1=sdg[:].rearrange("p (j c) -> p j c", c=2),
                    op=mybir.AluOpType.add)
                st2 = w_pool.tile([128, K * 2], F32, tag="st2")
                nc.vector.tensor_scalar_mul(out=st2[:], in0=st[:], scalar1=0.01)
                nc.vector.tensor_tensor(
                    out=st[:], in0=st[:], in1=st2[:], op=mybir.AluOpType.max)
                wt = w_pool.tile([128, K * 2], F32, tag="wt")
                nc.scalar.activation(
                    out=wt[:], in_=st[:], func=mybir.ActivationFunctionType.Exp)
                wt3 = wt[:].rearrange("p (j c) -> p j c", c=2)
                for h in range(H):
                    nc.vector.tensor_tensor(
                        out=zg3[:, :, h * OUT:(h + 1) * OUT],
                        in0=zg3[:, :, h * OUT:(h + 1) * OUT],
                        in1=wt3[:, :, h:h + 1].to_broadcast([128, K, OUT]),
                        op=mybir.AluOpType.mult)
                nc.vector.tensor_copy(out=zg3[:, :, IN:IN + 2], in_=wt3[:, :, :])
                ps = psB_pool.tile([128, ZC], F32)
                for j in range(K):
                    nc.tensor.matmul(
                        out=ps[:], lhsT=oht[:, j * 128:(j + 1) * 128],
                        rhs=zg[:, j * ZC:(j + 1) * ZC],
                        start=(j == 0), stop=(j == K - 1))
                den = f_pool.tile([128, 2], F32, tag="den")
                nc.vector.tensor_scalar_max(
                    out=den[:], in0=ps[:, IN:IN + 2], scalar1=1e-30)
                rec = f_pool.tile([128, 2], F32, tag="rec")
                nc.vector.reciprocal(out=rec[:], in_=den[:])
                o0 = f_pool.tile([128, OUT], F32, tag="o0")
                nc.vector.tensor_scalar_mul(
                    out=o0[:], in0=ps[:, 0:OUT], scalar1=rec[:, 0:1])
                ot = f_pool.tile([128, OUT], F32, tag="ot")
                nc.vector.tensor_scalar_mul(
                    out=ot[:], in0=ps[:, OUT:2 * OUT], scalar1=rec[:, 1:2])
                nc.vector.tensor_add(out=ot[:], in0=ot[:], in1=o0[:])
                nc.gpsimd.indirect_dma_start(
                    out=out[:, :],
                    out_offset=bass.IndirectOffsetOnAxis(
                        ap=ei[:, 3 * K:3 * K + 1], axis=0),
                    in_=ot[:], in_offset=None)

    nc.compile()
    return nc


def _prep_inputs(h_src, h_dst, W_src, W_dst, a_w, src_idx, dst_idx):
    """Host-side sharding/layout prep. Returns in_maps for the 8 cores."""
    hs = np.zeros((SRC_PAD, IN), np.float32)
    hs[:N_SRC] = h_src
    hsrcT = np.ascontiguousarray(hs.T.astype(ml_dtypes.bfloat16))

    # wsrc: [IN, ZC] = [ W[h,o,d] at col h*OUT+o | w~_s | pad ]
    wsr = np.zeros((IN, ZC), np.float32)
    wsr[:, :H * OUT] = W_src.reshape(H * OUT, IN).T
    a_s, a_d = a_w[:, :OUT], a_w[:, OUT:]
    wsr[:, H * OUT:H * OUT + H] = np.einsum("hod,ho->dh", W_src, a_s)
    wsr = wsr.astype(ml_dtypes.bfloat16)
    wds = np.einsum("hod,ho->dh", W_dst, a_d).astype(ml_dtypes.bfloat16)

    eidx, T = _pack_all(src_idx, dst_idx)

    in_maps = []
    for c in range(NCORES):
        hd = np.zeros((DST_PAD, IN), np.float32)
        hd[:NDST_C] = h_dst[c * NDST_C:(c + 1) * NDST_C]
        hdstT = np.ascontiguousarray(hd.T.astype(ml_dtypes.bfloat16))
        in_maps.append({
            "hsrcT": hsrcT,
            "hdstT": hdstT,
            "wsrc": wsr,
            "wdst": wds,
            "eidx": eidx[c],
        })
    return in_maps, T


def _run(inputs, trace=False):
    inputs = {k: np.asarray(v) for k, v in inputs.items()}
    in_maps, T = _prep_inputs(**inputs)
    nc = _build_program(T)
    res = run_bass_kernel_spmd(
        nc, in_maps, core_ids=list(range(NCORES)), trace=trace)
    parts = [res.results[c]["out"][:NDST_C] for c in range(NCORES)]
    return np.concatenate(parts, axis=0), res


def kernel(**inputs):
    out, _ = _run(inputs, trace=False)
    return out
